# revision 1
# baseline (speedup 1.0000x reference)
"""Decoder attention (QKV proj + KV-cache scatter + full softmax attention + out proj)
on 8 Trainium2 cores.

Sharding: core = (batch b, head-group g).  b = core//2, g = core%2; each core
handles 8 of the 16 heads for one batch element.

Key algorithmic point: softmax + attn@V are invariant to a permutation of the
key axis, so the reference's masked_scatter of new K/V into the cache is
equivalent to attending over concat([k_new, cache_keep]) where cache_keep are
the cache rows NOT in update_idx (complement set, gathered host-side during
sharding).  No on-device scatter is needed.

Device kernel (per core), all layouts chosen so no on-device transpose is
ever needed:
  - QKV:   qkT  (c', n)  = w_qkT.T @ xT      (c' = 8 q-heads*64 then 8 k-heads*64)
           v    (n,  c') = xT.T @ w_vT
  - attn:  scoresT (j, n) = k_eff.T-chunks @ qT ; exp on ACT (scale folded in);
           attn@V with V augmented by a ones-column -> softmax denominator
           accumulates for free in the same PSUM tile (row 64).
  - norm:  reciprocal + gpsimd partition_broadcast + DVE multiply.
  - proj:  outT (c_out, n) = w_projT.T @ attn_catT   (partial; host sums the
           two head-group partials per batch and adds b_proj).

All matmuls run in bf16 (fp32 PSUM accumulation): measured ~55us faster
than float32r on HW -- f32r streams slower than the cost model's 1
cycle/row; bf16 does not -- and input DMA bytes halve. Rel err 4.0e-3.
"""

import sys

import os

for _p in ("/opt/trn_rl_repo", "/root/.axon_site/_ro/trn_rl_repo"):
    if os.path.isdir(_p) and _p not in sys.path:
        sys.path.insert(0, _p)
        break

import numpy as np

import concourse.bacc as bacc
import concourse.mybir as mybir
import concourse.tile as tile
from concourse import bass_utils

B, NX, NC, C, H = 4, 1024, 2048, 1024, 16
DH = C // H                      # 64
G = 2                            # head groups (tensor-parallel factor)
HPG = H // G                     # 8 heads per group
CG = HPG * DH                    # 512 channels per group
SCALE = DH ** -0.5
N_CORES = 8
F32 = mybir.dt.float32
F32R = mybir.dt.float32r
BF16 = mybir.dt.bfloat16
EXP = mybir.ActivationFunctionType.Exp

# matmul dtypes per stage (float32r = full-rate, ~tf32 accuracy; accumulation
# is always fp32 in PSUM)
DT_QKV = F32R
DT_SCORES = F32R
DT_AV = F32R
DT_PROJ = F32R

_STATE = {}


def _r(ap, dt):
    return ap.bitcast(dt) if dt is not F32 else ap


def _build(reps: int = 1, exp_mode: str = "act"):
    """Build + compile the per-core Bass program.

    reps > 1 wraps the whole computation in an on-device hardware loop --
    used only for timing (amortizes host->device dispatch latency).
    """
    nc = bacc.Bacc("TRN2", target_bir_lowering=False, debug=False)

    xT_d = nc.dram_tensor("xT", [C, NX], BF16, kind="ExternalInput")
    wqkT_d = nc.dram_tensor("wqkT", [C, 2 * CG], BF16, kind="ExternalInput")
    wvT_d = nc.dram_tensor("wvT", [C, CG], BF16, kind="ExternalInput")
    bqk_d = nc.dram_tensor("bqk", [128, 8], F32, kind="ExternalInput")
    bv_d = nc.dram_tensor("bv", [128, CG], F32, kind="ExternalInput")
    kkeepT_d = nc.dram_tensor("kkeepT", [CG, NC - NX], BF16, kind="ExternalInput")
    vkeep_d = nc.dram_tensor("vkeep", [NC - NX, HPG * (DH + 1)], BF16, kind="ExternalInput")
    wprojT_d = nc.dram_tensor("wprojT", [CG, C], BF16, kind="ExternalInput")
    ones_d = nc.dram_tensor("ones8", [128, 8], BF16, kind="ExternalInput")
    outT_d = nc.dram_tensor("outT", [C, NX], BF16, kind="ExternalOutput")

    NJ = NC // 128               # 16 j-tiles over the effective kv length
    VW = DH + 1                  # 65: v columns + ones column per head

    with tile.TileContext(nc) as tc:
        with (
            tc.tile_pool(name="persist", bufs=1) as pp,
            tc.tile_pool(name="work", bufs=1) as wp,
            tc.tile_pool(name="wqkc", bufs=4) as wqkp,
            tc.tile_pool(name="attn", bufs=4) as ep,
            tc.tile_pool(name="nrm", bufs=2) as np_pool,
            tc.tile_pool(name="out_sb", bufs=1) as op,
            tc.tile_pool(name="ps", bufs=1, space="PSUM") as psp,
        ):
            # ---- persistent tiles ----
            q_t = [pp.tile([128, NX], BF16, tag=f"q{i}", name=f"q{i}") for i in range(4)]
            k_t = [pp.tile([128, NC], BF16, tag=f"k{i}", name=f"k{i}") for i in range(4)]
            v_t = [pp.tile([128, HPG * VW], BF16, tag=f"v{i}", name=f"v{i}") for i in range(NJ)]
            a_t = [pp.tile([128, NX], BF16, tag=f"a{i}", name=f"a{i}") for i in range(4)]
            bqk_t = pp.tile([128, 8], F32, tag="bqk")
            bv_t = pp.tile([128, CG], F32, tag="bv")
            xT_t = [wp.tile([128, NX], BF16, tag=f"x{i}", name=f"x{i}") for i in range(8)]
            wv_t = [wp.tile([128, CG], BF16, tag=f"wv{i}", name=f"wv{i}") for i in range(8)]

            nc.sync.dma_start(bqk_t[:], bqk_d.ap())
            nc.sync.dma_start(bv_t[:], bv_d.ap())

            def body():
                # ACT exp-table preload: a tiny dummy exp up front makes the
                # ~2.7us ACT_TABLE_LOAD happen during the DMA-bound prologue
                # instead of delaying the first real softmax exp.
                scr0 = wp.tile([128, 8], F32, tag="scr0")
                scr1 = wp.tile([128, 8], F32, tag="scr1")
                nc.vector.memset(scr0[:], 0.0)
                nc.scalar.activation(scr1[:], scr0[:], EXP)

                # ---- priority DMAs, ordered by first use.  The m=0/m=4
                # weight batches go out FIRST (right after xT chunk 0) so the
                # prologue qk matmuls are paced by the xT stream, not stuck
                # behind ~5MB of v/k-cache input traffic.
                nc.sync.dma_start(xT_t[0][:], xT_d[0:128, :])
                wqk_pre = {}
                for m in (0, 4):
                    wqk_pre[m] = wqkp.tile([128, 1024], BF16, tag="wqkc",
                                           bufs=3, name=f"wqkm{m}")
                    nc.sync.dma_start(
                        wqk_pre[m][:].rearrange("p (kk c) -> p kk c", kk=8),
                        wqkT_d[0:C, m * 128:(m + 1) * 128]
                        .rearrange("(kk p) c -> p kk c", p=128),
                    )
                nc.sync.dma_start(k_t[0][:, NX:NC], kkeepT_d[0:128, :])
                for i in range(1, 8):
                    nc.sync.dma_start(xT_t[i][:], xT_d[i * 128:(i + 1) * 128, :])
                # ones columns of the v tiles (bias adds never touch them;
                # vkeep rows arrive with ones baked in)
                for m in range(NJ // 2):
                    nc.sync.dma_start(
                        v_t[m][:].rearrange("p (h w) -> p h w", w=VW)[:, :, DH],
                        ones_d.ap(),
                    )
                for i in range(8):
                    nc.sync.dma_start(wv_t[i][:], wvT_d[i * 128:(i + 1) * 128, :])
                for i in range(1, 4):
                    nc.sync.dma_start(k_t[i][:, NX:NC], kkeepT_d[i * 128:(i + 1) * 128, :])
                for j in range(NJ // 2, NJ):
                    r0 = (j - NJ // 2) * 128
                    nc.sync.dma_start(v_t[j][:], vkeep_d[r0:r0 + 128, :])

                def qk_thunks(i):
                    """Matmul/bias thunks for q m-tile i and k m-tile 4+i,
                    drained one per attention j-step.  The m-tile's 8 weight
                    K-chunks arrive in ONE batched strided DMA (pre-issued
                    for the prologue pair)."""
                    for m in (i, 4 + i):
                        qps = psp.tile([128, NX], F32, tag="qps", bufs=1, name=f"qps{m}")
                        if m in wqk_pre:
                            wqk_m = wqk_pre[m]
                        else:
                            wqk_m = wqkp.tile([128, 1024], BF16, tag="wqkc",
                                              bufs=3, name=f"wqkm{m}")

                            def wdma(m=m, wqk_m=wqk_m):
                                nc.sync.dma_start(
                                    wqk_m[:].rearrange("p (kk c) -> p kk c",
                                                       kk=8),
                                    wqkT_d[0:C, m * 128:(m + 1) * 128]
                                    .rearrange("(kk p) c -> p kk c", p=128),
                                )
                            yield wdma
                        for kk in range(8):
                            def mm(m=m, kk=kk, qps=qps, wqk_m=wqk_m):
                                for cch in range(2):
                                    nc.tensor.matmul(
                                        qps[:, cch * 512:(cch + 1) * 512],
                                        wqk_m[:, kk * 128:(kk + 1) * 128],
                                        xT_t[kk][:, cch * 512:(cch + 1) * 512],
                                        start=(kk == 0),
                                        stop=(kk == 7),
                                    )
                            yield mm
                        def bias(m=m, qps=qps):
                            if m < 4:
                                dest = q_t[m][:]
                            else:
                                dest = k_t[m - 4][:, 0:NX]
                            nc.vector.tensor_scalar_add(dest, qps[:], bqk_t[:, m:m + 1])
                        yield bias

                pending = []

                def drain(n):
                    for _ in range(n):
                        if not pending:
                            return
                        pending.pop(0)()

                # qk pair 0 runs up front (attention depends on it)
                for th in qk_thunks(0):
                    th()

                def v_thunks():
                    """v projection m-tiles as drain thunks (9 per m: 8 matmuls
                    + the bias/scatter finisher), interleaved into head 0."""
                    for m in range(8):
                        vps = psp.tile([128, NX], F32, tag="qps", bufs=1, name=f"vps{m}")
                        for kk in range(8):
                            def mm(m=m, kk=kk, vps=vps):
                                nc.tensor.matmul(
                                    vps[:, 0:CG],
                                    xT_t[kk][:, m * 128:(m + 1) * 128],
                                    wv_t[kk][:],
                                    start=(kk == 0),
                                    stop=(kk == 7),
                                )
                            yield mm
                        def fin(m=m, vps=vps):
                            for h in range(HPG):
                                nc.vector.tensor_add(
                                    v_t[m][:, h * VW:h * VW + DH],
                                    vps[:, h * DH:(h + 1) * DH],
                                    bv_t[:, h * DH:(h + 1) * DH],
                                )
                        yield fin

                # ---- phase 2: attention; j loop software-pipelined (av for
                # j-1 after scores/exp for j) with leftover QKV matmuls
                # drained one per j-step to fill PE idle time ----
                def attn_head(h, av_lag=1, drain_per_j=1, j_order=None):
                    hp, po = h // 2, 64 * (h % 2)
                    jo = list(j_order) if j_order is not None else list(range(NJ))
                    av = psp.tile([VW, NX], F32, tag="av", bufs=1, name=f"av{h}")
                    ets = [None] * NJ

                    def emit_av(j):
                        for cch in range(2):
                            nc.tensor.matmul(
                                av[:, cch * 512:(cch + 1) * 512],
                                v_t[j][:, h * VW:(h + 1) * VW],
                                ets[j][:, cch * 512:(cch + 1) * 512],
                                start=(j == jo[0]),
                                stop=(j == jo[-1]),
                            )

                    for step, j in enumerate(jo):
                        sps = psp.tile([128, NX], F32, tag="sps", bufs=2, name=f"sps{h}_{j}")
                        for cch in range(2):
                            nc.tensor.matmul(
                                sps[:, cch * 512:(cch + 1) * 512],
                                k_t[hp][po:po + 64, j * 128:(j + 1) * 128],
                                q_t[hp][po:po + 64, cch * 512:(cch + 1) * 512],
                                start=True,
                                stop=True,
                            )
                        et = ep.tile([128, NX], BF16, tag="et", name=f"et{h}_{j}")
                        ets[j] = et
                        if exp_mode == "act":
                            nc.scalar.activation(et[:], sps[:], EXP, scale=SCALE)
                        else:
                            # timing probe only: wrong math, same data movement
                            nc.vector.tensor_copy(et[:], sps[:])
                        drain(drain_per_j)
                        if step >= av_lag:
                            emit_av(jo[step - av_lag])
                    for step in range(NJ - av_lag, NJ):
                        emit_av(jo[step])
                    avs = np_pool.tile([VW, NX], F32, tag="avs", bufs=2, name=f"avs{h}")
                    nc.vector.tensor_copy(avs[:], av[:])
                    recip = np_pool.tile([1, NX], F32, tag="recip", bufs=1)
                    nc.vector.reciprocal(recip[:], avs[DH:VW, :])
                    rb = np_pool.tile([64, NX], F32, tag="rb", bufs=1)
                    nc.gpsimd.partition_broadcast(rb[:], recip[:])
                    nc.vector.tensor_mul(a_t[hp][po:po + 64, :], avs[0:DH, :], rb[:])

                pending.extend(v_thunks())
                # head 0 visits the cache-half key tiles first: their V rows
                # come from DMA, so attention starts before the V projection
                # (draining concurrently) has produced anything
                attn_head(0, av_lag=2, drain_per_j=9,
                          j_order=list(range(NJ // 2, NJ)) + list(range(NJ // 2)))
                for h in range(1, HPG):
                    if h in (2, 4, 6):
                        drain(80)            # pair (h//2) must be complete
                    if h in (1, 3, 5):
                        pending.extend(qk_thunks((h + 1) // 2))
                    attn_head(h)
                drain(80)

                # ---- phase 3: output projection (partial: this head group);
                # w_proj column-chunks streamed per m like w_qk ----
                for m in range(8):
                    pps = psp.tile([128, NX], F32, tag="sps", bufs=2, name=f"pps{m}")
                    wp_m = op.tile([128, 512], BF16, tag="wpc", bufs=2,
                                   name=f"wpm{m}")
                    nc.sync.dma_start(
                        wp_m[:].rearrange("p (kk c) -> p kk c", kk=4),
                        wprojT_d[0:CG, m * 128:(m + 1) * 128]
                        .rearrange("(kk p) c -> p kk c", p=128),
                    )
                    for cch in range(2):
                        for kk in range(4):
                            nc.tensor.matmul(
                                pps[:, cch * 512:(cch + 1) * 512],
                                wp_m[:, kk * 128:(kk + 1) * 128],
                                a_t[kk][:, cch * 512:(cch + 1) * 512],
                                start=(kk == 0),
                                stop=(kk == 3),
                            )
                    ot = op.tile([128, NX], BF16, tag="ot", bufs=2)
                    if m % 2 == 0:
                        nc.vector.tensor_copy(ot[:], pps[:])
                    else:
                        # ACT is idle during proj; split PSUM evacuation
                        nc.scalar.activation(
                            ot[:], pps[:], mybir.ActivationFunctionType.Identity
                        )
                    nc.sync.dma_start(outT_d[m * 128:(m + 1) * 128, :], ot[:])

            if reps == 1:
                body()
            else:
                hints = (
                    mybir.EngineType.PE,
                    mybir.EngineType.Activation,
                    mybir.EngineType.DVE,
                    mybir.EngineType.SP,
                )
                with tc.For_i(0, reps, 1, hint_engines=hints):
                    body()

    nc.compile()
    return nc


def _get_nc():
    if "nc" not in _STATE:
        _STATE["nc"] = _build()
    return _STATE["nc"]


def _prep_in_maps(x, update_idx, cache_k, cache_v, w_qkv, b_qkv):
    """Host-side sharding: build the 8 per-core input dicts."""
    x = np.asarray(x, np.float32)
    update_idx = np.asarray(update_idx)
    cache_k = np.asarray(cache_k, np.float32)
    cache_v = np.asarray(cache_v, np.float32)
    w_qkv = np.asarray(w_qkv, np.float32)
    b_qkv = np.asarray(b_qkv, np.float32)

    per_g = []
    for g in range(G):
        qs = slice(g * CG, (g + 1) * CG)
        ks = slice(C + g * CG, C + (g + 1) * CG)
        vs = slice(2 * C + g * CG, 2 * C + (g + 1) * CG)
        import ml_dtypes
        wqkT = np.ascontiguousarray(
            np.concatenate([w_qkv[qs], w_qkv[ks]], 0).T
        ).astype(ml_dtypes.bfloat16)                                                    # (C, 2CG)
        wvT = np.ascontiguousarray(w_qkv[vs].T).astype(ml_dtypes.bfloat16)              # (C, CG)
        bqk = np.ascontiguousarray(
            np.concatenate([b_qkv[qs], b_qkv[ks]]).reshape(8, 128).T
        )                                                    # (128, 8)
        bv = np.broadcast_to(b_qkv[vs][None, :], (128, CG)).copy()
        per_g.append((wqkT, wvT, bqk, bv))

    in_maps = []
    for b in range(B):
        idx = update_idx[b]
        mask = np.ones(NC, bool)
        mask[idx] = False
        keep = np.nonzero(mask)[0]                           # (NC-NX,) sorted
        xT = np.ascontiguousarray(x[b].T).astype(__import__("ml_dtypes").bfloat16)                    # (C, NX)
        for g in range(G):
            wqkT, wvT, bqk, bv = per_g[g]
            hsel = slice(g * HPG, (g + 1) * HPG)
            kk = cache_k[b, hsel][:, keep, :]                # (HPG, NC-NX, DH)
            import ml_dtypes
            kkeepT = np.ascontiguousarray(
                kk.transpose(0, 2, 1).reshape(HPG * DH, NC - NX)
            ).astype(ml_dtypes.bfloat16)
            vk = cache_v[b, hsel][:, keep, :].transpose(1, 0, 2)  # (NC-NX, HPG, DH)
            vkeep = np.ascontiguousarray(
                np.concatenate(
                    [vk, np.ones((NC - NX, HPG, 1), np.float32)], axis=2
                ).reshape(NC - NX, HPG * (DH + 1))
            ).astype(ml_dtypes.bfloat16)
            wprojT = np.asarray(_STATE["wprojT"][g], __import__("ml_dtypes").bfloat16)
            in_maps.append(
                dict(
                    xT=xT, wqkT=wqkT, wvT=wvT, bqk=bqk, bv=bv,
                    kkeepT=kkeepT, vkeep=vkeep, wprojT=wprojT,
                    ones8=np.ones((128, 8), __import__('ml_dtypes').bfloat16),
                )
            )
    return in_maps


def kernel(x, update_idx, cache_k, cache_v, w_qkv, b_qkv, w_proj, b_proj):
    nc = _get_nc()
    w_proj = np.asarray(w_proj, np.float32)
    b_proj = np.asarray(b_proj, np.float32)
    _STATE["wprojT"] = [
        np.ascontiguousarray(w_proj[:, g * CG:(g + 1) * CG].T) for g in range(G)
    ]
    in_maps = _prep_in_maps(x, update_idx, cache_k, cache_v, w_qkv, b_qkv)
    res = bass_utils.run_bass_kernel_spmd(nc, in_maps, core_ids=list(range(N_CORES)))
    _STATE["last_results"] = res
    out = np.empty((B, NX, C), np.float32)
    for b in range(B):
        acc = (res.results[2 * b]["outT"].astype(np.float32)
               + res.results[2 * b + 1]["outT"].astype(np.float32))
        out[b] = acc.T + b_proj
    return out



# revision 4
# speedup vs baseline: 1.1364x; 1.1364x over previous
"""Decoder attention (QKV proj + KV-cache scatter + full softmax attention + out proj)
on 8 Trainium2 cores.

Sharding: core = (batch b, head-group g).  b = core//2, g = core%2; each core
handles 8 of the 16 heads for one batch element.

Key algorithmic point: softmax + attn@V are invariant to a permutation of the
key axis, so the reference's masked_scatter of new K/V into the cache is
equivalent to attending over concat([k_new, cache_keep]) where cache_keep are
the cache rows NOT in update_idx (complement set, gathered host-side during
sharding).  No on-device scatter is needed.

v2 structure (HW-microbenchmarked):
  - scores run with the contraction PADDED to K=128 (zeros in the k tiles).
    Measured: K=64 matmuls stream at 2 cycles/row on TRN2; K>=96 at 1.
    Each per-head k tile [128, NC] holds its 64 dh rows at the head's own
    partition offset (po) and zeros elsewhere, so the full 128-partition q
    tile (2 heads stacked) can be streamed unmodified as the moving operand.
  - attn@V is TRANSPOSED: out[n,d] chunks [128, 65] with et as stationary and
    the per-head v slice [128 kv, 64+ones] as the 65-row moving operand.
    Measured 279ns vs 435ns per (head, j-tile).  The ones column makes the
    softmax denominator accumulate in column 64 of each chunk.
  - normalization is then per-partition scalars (DVE reciprocal +
    tensor_scalar mult), no partition_broadcast needed.
  - the [n, cg] attention output is PE-transposed (identity moving operand)
    back to [cg, n] for the output projection, 128x128 blocks, interleaved
    into the attention drain stream.
  - fp8/DoubleRow: measured NO gain over bf16 (sc_fp8dr == sc_bf16), not used.

All matmuls bf16 (fp32 PSUM accumulation).
"""

import sys

import os

for _p in ("/opt/trn_rl_repo", "/root/.axon_site/_ro/trn_rl_repo"):
    if os.path.isdir(_p) and _p not in sys.path:
        sys.path.insert(0, _p)
        break

import numpy as np

import concourse.bacc as bacc
import concourse.mybir as mybir
import concourse.tile as tile
from concourse import bass_utils

B, NX, NC, C, H = 4, 1024, 2048, 1024, 16
DH = C // H                      # 64
G = 2                            # head groups (tensor-parallel factor)
HPG = H // G                     # 8 heads per group
CG = HPG * DH                    # 512 channels per group
SCALE = DH ** -0.5
N_CORES = 8
F32 = mybir.dt.float32
BF16 = mybir.dt.bfloat16
EXP = mybir.ActivationFunctionType.Exp
IDENT = mybir.ActivationFunctionType.Identity

_STATE = {}


def _build(reps: int = 1, exp_mode: str = "act"):
    """Build + compile the per-core Bass program.

    reps > 1 wraps the whole computation in an on-device hardware loop --
    used only for timing (amortizes host->device dispatch latency).
    """
    nc = bacc.Bacc("TRN2", target_bir_lowering=False, debug=False)

    xT_d = nc.dram_tensor("xT", [C, NX], BF16, kind="ExternalInput")
    wqkT_d = nc.dram_tensor("wqkT", [C, 2 * CG], BF16, kind="ExternalInput")
    wvT_d = nc.dram_tensor("wvT", [C, CG], BF16, kind="ExternalInput")
    bqk_d = nc.dram_tensor("bqk", [128, 8], F32, kind="ExternalInput")
    bv_d = nc.dram_tensor("bv", [128, CG], F32, kind="ExternalInput")
    kkeepT_d = nc.dram_tensor("kkeepT", [CG, NC - NX], BF16, kind="ExternalInput")
    vkeep_d = nc.dram_tensor("vkeep", [NC - NX, HPG * (DH + 1)], BF16, kind="ExternalInput")
    wprojT_d = nc.dram_tensor("wprojT", [CG, C], BF16, kind="ExternalInput")
    ones_d = nc.dram_tensor("ones8", [128, 8], BF16, kind="ExternalInput")
    id_d = nc.dram_tensor("id128", [128, 128], BF16, kind="ExternalInput")
    outT_d = nc.dram_tensor("outT", [C, NX], BF16, kind="ExternalOutput")

    NJ = NC // 128               # 16 j-tiles over the effective kv length
    VW = DH + 1                  # 65: v columns + ones column per head
    NT = NX // 128               # 8 query n-tiles

    with tile.TileContext(nc) as tc:
        with (
            tc.tile_pool(name="persist", bufs=1) as pp,
            tc.tile_pool(name="work", bufs=1) as wp,
            tc.tile_pool(name="wqkc", bufs=4) as wqkp,
            tc.tile_pool(name="attn", bufs=4) as ep,
            tc.tile_pool(name="nrm", bufs=2) as np_pool,
            tc.tile_pool(name="out_sb", bufs=1) as op,
            tc.tile_pool(name="ps", bufs=1, space="PSUM") as psp,
        ):
            # ---- persistent tiles ----
            q_t = [pp.tile([128, NX], BF16, tag=f"q{i}", name=f"q{i}") for i in range(4)]
            # per-head k tiles, dh rows at partition offset po(h), zeros
            # elsewhere (K=128 padding for full-rate scores matmuls)
            k_t = [pp.tile([128, NC], BF16, tag=f"k{h}", name=f"k{h}") for h in range(HPG)]
            v_t = [pp.tile([128, HPG * VW], BF16, tag=f"v{i}", name=f"v{i}") for i in range(NJ)]
            # attention out, transposed layout: aT_t[nt] = [128 n, CG] bf16
            aT_t = [pp.tile([128, CG], BF16, tag=f"aT{i}", name=f"aT{i}") for i in range(NT)]
            # attention out, proj layout: a_t[c] = [128 cg-chunk, NX]
            a_t = [pp.tile([128, NX], BF16, tag=f"a{i}", name=f"a{i}") for i in range(4)]
            id_t = pp.tile([128, 128], BF16, tag="id128")
            bqk_t = pp.tile([128, 8], F32, tag="bqk")
            bv_t = pp.tile([128, CG], F32, tag="bv")
            xT_t = [wp.tile([128, NX], BF16, tag=f"x{i}", name=f"x{i}") for i in range(8)]
            wv_t = [wp.tile([128, CG], BF16, tag=f"wv{i}", name=f"wv{i}") for i in range(8)]

            # zero the k pad rows once (outside the timing loop; the loop only
            # ever rewrites the dh rows, so the pad stays zero across reps)
            for h in range(HPG):
                po = DH * (h % 2)
                pad0 = 0 if h % 2 == 1 else DH
                nc.vector.memset(k_t[h][pad0:pad0 + DH, :], 0.0)

            nc.sync.dma_start(bqk_t[:], bqk_d.ap())
            nc.sync.dma_start(bv_t[:], bv_d.ap())
            nc.sync.dma_start(id_t[:], id_d.ap())

            def body():
                # ACT exp-table preload: a tiny dummy exp up front makes the
                # ~2.7us ACT_TABLE_LOAD happen during the DMA-bound prologue
                # instead of delaying the first real softmax exp.
                scr0 = wp.tile([128, 8], F32, tag="scr0")
                scr1 = wp.tile([128, 8], F32, tag="scr1")
                nc.vector.memset(scr0[:], 0.0)
                nc.scalar.activation(scr1[:], scr0[:], EXP)

                # ---- priority DMAs, ordered by first use.  The m=0/m=4
                # weight batches go out FIRST (right after xT chunk 0) so the
                # prologue qk matmuls are paced by the xT stream, not stuck
                # behind ~5MB of v/k-cache input traffic.
                nc.sync.dma_start(xT_t[0][:], xT_d[0:128, :])
                wqk_pre = {}
                for m in (0, 4):
                    wqk_pre[m] = wqkp.tile([128, 1024], BF16, tag="wqkc",
                                           bufs=3, name=f"wqkm{m}")
                    nc.sync.dma_start(
                        wqk_pre[m][:].rearrange("p (kk c) -> p kk c", kk=8),
                        wqkT_d[0:C, m * 128:(m + 1) * 128]
                        .rearrange("(kk p) c -> p kk c", p=128),
                    )
                # kkeep for heads 0,1 (head h dh rows at partition offset po)
                for h in (0, 1):
                    po = DH * (h % 2)
                    nc.sync.dma_start(
                        k_t[h][po:po + DH, NX:NC],
                        kkeepT_d[h * DH:(h + 1) * DH, :],
                    )
                for i in range(1, 8):
                    nc.sync.dma_start(xT_t[i][:], xT_d[i * 128:(i + 1) * 128, :])
                # ones columns of the v tiles (bias adds never touch them;
                # vkeep rows arrive with ones baked in)
                for m in range(NJ // 2):
                    nc.sync.dma_start(
                        v_t[m][:].rearrange("p (h w) -> p h w", w=VW)[:, :, DH],
                        ones_d.ap(),
                    )
                for i in range(8):
                    nc.sync.dma_start(wv_t[i][:], wvT_d[i * 128:(i + 1) * 128, :])
                for h in range(2, HPG):
                    po = DH * (h % 2)
                    nc.sync.dma_start(
                        k_t[h][po:po + DH, NX:NC],
                        kkeepT_d[h * DH:(h + 1) * DH, :],
                    )
                for j in range(NJ // 2, NJ):
                    r0 = (j - NJ // 2) * 128
                    nc.sync.dma_start(v_t[j][:], vkeep_d[r0:r0 + 128, :])

                def qk_thunks(i):
                    """Matmul/bias thunks for q m-tile i and k m-tile 4+i,
                    drained one per attention j-step.  qps is a single PSUM
                    bank [128, 512]; each 512-wide cch chunk is its own
                    matmul chain + DVE evacuation."""
                    for m in (i, 4 + i):
                        if m in wqk_pre:
                            wqk_m = wqk_pre[m]
                        else:
                            wqk_m = wqkp.tile([128, 1024], BF16, tag="wqkc",
                                              bufs=3, name=f"wqkm{m}")

                            def wdma(m=m, wqk_m=wqk_m):
                                nc.sync.dma_start(
                                    wqk_m[:].rearrange("p (kk c) -> p kk c",
                                                       kk=8),
                                    wqkT_d[0:C, m * 128:(m + 1) * 128]
                                    .rearrange("(kk p) c -> p kk c", p=128),
                                )
                            yield wdma
                        for cch in range(2):
                            qps = psp.tile([128, 512], F32, tag="qps", bufs=1,
                                           name=f"qps{m}_{cch}")
                            for kk in range(8):
                                def mm(m=m, kk=kk, cch=cch, qps=qps, wqk_m=wqk_m):
                                    nc.tensor.matmul(
                                        qps[:],
                                        wqk_m[:, kk * 128:(kk + 1) * 128],
                                        xT_t[kk][:, cch * 512:(cch + 1) * 512],
                                        start=(kk == 0),
                                        stop=(kk == 7),
                                    )
                                yield mm
                            def bias(m=m, cch=cch, qps=qps):
                                cs = slice(cch * 512, (cch + 1) * 512)
                                if m < 4:
                                    nc.vector.tensor_scalar_add(
                                        q_t[m][:, cs], qps[:], bqk_t[:, m:m + 1])
                                else:
                                    i2 = m - 4
                                    # two heads: dh rows land at each head's po
                                    nc.vector.tensor_scalar_add(
                                        k_t[2 * i2][0:DH, cs], qps[0:DH, :],
                                        bqk_t[0:DH, m:m + 1])
                                    nc.vector.tensor_scalar_add(
                                        k_t[2 * i2 + 1][DH:128, cs], qps[DH:128, :],
                                        bqk_t[DH:128, m:m + 1])
                            yield bias

                pending = []

                def drain(n):
                    for _ in range(n):
                        if not pending:
                            return
                        pending.pop(0)()

                # qk pair 0 runs up front (attention depends on it)
                for th in qk_thunks(0):
                    th()

                def v_thunks():
                    """v projection m-tiles as drain thunks (9 per m: 8 matmuls
                    + one batched strided bias/scatter finisher)."""
                    for m in range(8):
                        vps = psp.tile([128, 512], F32, tag="qps", bufs=1,
                                       name=f"vps{m}")
                        for kk in range(8):
                            def mm(m=m, kk=kk, vps=vps):
                                nc.tensor.matmul(
                                    vps[:],
                                    xT_t[kk][:, m * 128:(m + 1) * 128],
                                    wv_t[kk][:],
                                    start=(kk == 0),
                                    stop=(kk == 7),
                                )
                            yield mm
                        def fin(m=m, vps=vps):
                            nc.vector.tensor_add(
                                v_t[m][:].rearrange("p (h w) -> p h w", w=VW)
                                [:, :, 0:DH],
                                vps[:].rearrange("p (h w) -> p h w", w=DH),
                                bv_t[:].rearrange("p (h w) -> p h w", w=DH),
                            )
                        yield fin

                def transpose_thunks(c):
                    """PE-transpose aT_t[:, c*128:(c+1)*128] -> a_t[c], in two
                    4-block batches through one PSUM bank."""
                    for half in range(2):
                        trp = psp.tile([128, 512], BF16, tag="tr", bufs=1,
                                       name=f"tr{c}_{half}")
                        for q in range(4):
                            nt = half * 4 + q
                            def tmm(c=c, q=q, nt=nt, trp=trp):
                                nc.tensor.matmul(
                                    trp[:, q * 128:(q + 1) * 128],
                                    aT_t[nt][:, c * 128:(c + 1) * 128],
                                    id_t[:],
                                    start=(q == 0), stop=(q == 3),
                                    is_transpose=True,
                                )
                            yield tmm
                        def tev(c=c, half=half, trp=trp):
                            nc.vector.tensor_copy(
                                a_t[c][:, half * 512:(half + 1) * 512], trp[:])
                        yield tev

                # ---- phase 2: attention; per j-step: scores (K=128 padded),
                # exp on ACT, transposed av for the previous j (8 chunk
                # matmuls), plus drained QKV/transpose thunks to fill PE ----
                def attn_head(h, av_lag=1, drain_per_j=1, j_order=None):
                    hp = h // 2
                    jo = list(j_order) if j_order is not None else list(range(NJ))
                    avA = psp.tile([128, 4 * VW], F32, tag="avA", bufs=1,
                                   name=f"avA{h}")
                    avB = psp.tile([128, 4 * VW], F32, tag="avB", bufs=1,
                                   name=f"avB{h}")
                    ets = [None] * NJ

                    def emit_av(j):
                        # one accumulation group per PSUM bank (2KB zero
                        # region): start only on the bank's first write,
                        # stop on its last; per-element first-touch handles
                        # the four disjoint nt slices within the bank.
                        for nt in range(NT):
                            dst = avA if nt < 4 else avB
                            o = (nt % 4) * VW
                            nc.tensor.matmul(
                                dst[:, o:o + VW],
                                ets[j][:, nt * 128:(nt + 1) * 128],
                                v_t[j][:, h * VW:(h + 1) * VW],
                                start=(j == jo[0] and nt % 4 == 0),
                                stop=(j == jo[-1] and nt % 4 == 3),
                            )

                    for step, j in enumerate(jo):
                        sps = psp.tile([128, NX], F32, tag="sps", bufs=2, name=f"sps{h}_{j}")
                        for cch in range(2):
                            nc.tensor.matmul(
                                sps[:, cch * 512:(cch + 1) * 512],
                                k_t[h][:, j * 128:(j + 1) * 128],
                                q_t[hp][:, cch * 512:(cch + 1) * 512],
                                start=True,
                                stop=True,
                            )
                        et = ep.tile([128, NX], BF16, tag="et", name=f"et{h}_{j}")
                        ets[j] = et
                        if exp_mode == "act":
                            nc.scalar.activation(et[:], sps[:], EXP, scale=SCALE)
                        else:
                            # timing probe only: wrong math, same data movement
                            nc.vector.tensor_copy(et[:], sps[:])
                        drain(drain_per_j)
                        if step >= av_lag:
                            emit_av(jo[step - av_lag])
                    for step in range(NJ - av_lag, NJ):
                        emit_av(jo[step])
                    # normalization: per n-tile, denominator is column 64 of
                    # the av chunk; divide via per-partition scalars on DVE.
                    for nt in range(NT):
                        src = avA if nt < 4 else avB
                        o = (nt % 4) * VW
                        rc = np_pool.tile([128, 1], F32, tag="rc", bufs=2,
                                          name=f"rc{h}_{nt}")
                        nc.vector.reciprocal(rc[:], src[:, o + DH:o + DH + 1])
                        nc.vector.tensor_scalar_mul(
                            aT_t[nt][:, h * DH:(h + 1) * DH],
                            src[:, o:o + DH], rc[:])

                pending.extend(v_thunks())
                # head 0 visits the cache-half key tiles first: their V rows
                # come from DMA, so attention starts before the V projection
                # (draining concurrently) has produced anything
                attn_head(0, av_lag=2, drain_per_j=9,
                          j_order=list(range(NJ // 2, NJ)) + list(range(NJ // 2)))
                for h in range(1, HPG):
                    if h in (2, 4, 6):
                        drain(90)            # pair (h//2) must be complete
                    if h in (1, 3, 5):
                        pending.extend(qk_thunks((h + 1) // 2))
                    if h in (2, 4, 6):
                        pending.extend(transpose_thunks(h // 2 - 1))
                    attn_head(h)
                pending.extend(transpose_thunks(3))
                drain(200)

                # ---- phase 3: output projection (partial: this head group);
                # w_proj column-chunks streamed per m like w_qk ----
                for m in range(8):
                    pps = psp.tile([128, NX], F32, tag="sps", bufs=2, name=f"pps{m}")
                    wp_m = op.tile([128, 512], BF16, tag="wpc", bufs=2,
                                   name=f"wpm{m}")
                    nc.sync.dma_start(
                        wp_m[:].rearrange("p (kk c) -> p kk c", kk=4),
                        wprojT_d[0:CG, m * 128:(m + 1) * 128]
                        .rearrange("(kk p) c -> p kk c", p=128),
                    )
                    for cch in range(2):
                        for kk in range(4):
                            nc.tensor.matmul(
                                pps[:, cch * 512:(cch + 1) * 512],
                                wp_m[:, kk * 128:(kk + 1) * 128],
                                a_t[kk][:, cch * 512:(cch + 1) * 512],
                                start=(kk == 0),
                                stop=(kk == 3),
                            )
                    ot = op.tile([128, NX], BF16, tag="ot", bufs=2)
                    if m % 2 == 0:
                        nc.vector.tensor_copy(ot[:], pps[:])
                    else:
                        # ACT is idle during proj; split PSUM evacuation
                        nc.scalar.activation(ot[:], pps[:], IDENT)
                    nc.sync.dma_start(outT_d[m * 128:(m + 1) * 128, :], ot[:])

            if reps == 1:
                body()
            else:
                hints = (
                    mybir.EngineType.PE,
                    mybir.EngineType.Activation,
                    mybir.EngineType.DVE,
                    mybir.EngineType.SP,
                )
                with tc.For_i(0, reps, 1, hint_engines=hints):
                    body()

    nc.compile()
    return nc


def _get_nc():
    if "nc" not in _STATE:
        _STATE["nc"] = _build()
    return _STATE["nc"]


def _prep_in_maps(x, update_idx, cache_k, cache_v, w_qkv, b_qkv):
    """Host-side sharding: build the 8 per-core input dicts."""
    import ml_dtypes

    x = np.asarray(x, np.float32)
    update_idx = np.asarray(update_idx)
    cache_k = np.asarray(cache_k, np.float32)
    cache_v = np.asarray(cache_v, np.float32)
    w_qkv = np.asarray(w_qkv, np.float32)
    b_qkv = np.asarray(b_qkv, np.float32)

    per_g = []
    for g in range(G):
        qs = slice(g * CG, (g + 1) * CG)
        ks = slice(C + g * CG, C + (g + 1) * CG)
        vs = slice(2 * C + g * CG, 2 * C + (g + 1) * CG)
        wqkT = np.ascontiguousarray(
            np.concatenate([w_qkv[qs], w_qkv[ks]], 0).T
        ).astype(ml_dtypes.bfloat16)                                                    # (C, 2CG)
        wvT = np.ascontiguousarray(w_qkv[vs].T).astype(ml_dtypes.bfloat16)              # (C, CG)
        bqk = np.ascontiguousarray(
            np.concatenate([b_qkv[qs], b_qkv[ks]]).reshape(8, 128).T
        )                                                    # (128, 8)
        bv = np.broadcast_to(b_qkv[vs][None, :], (128, CG)).copy()
        per_g.append((wqkT, wvT, bqk, bv))

    in_maps = []
    for b in range(B):
        idx = update_idx[b]
        mask = np.ones(NC, bool)
        mask[idx] = False
        keep = np.nonzero(mask)[0]                           # (NC-NX,) sorted
        xT = np.ascontiguousarray(x[b].T).astype(ml_dtypes.bfloat16)                    # (C, NX)
        for g in range(G):
            wqkT, wvT, bqk, bv = per_g[g]
            hsel = slice(g * HPG, (g + 1) * HPG)
            kk = cache_k[b, hsel][:, keep, :]                # (HPG, NC-NX, DH)
            kkeepT = np.ascontiguousarray(
                kk.transpose(0, 2, 1).reshape(HPG * DH, NC - NX)
            ).astype(ml_dtypes.bfloat16)
            vk = cache_v[b, hsel][:, keep, :].transpose(1, 0, 2)  # (NC-NX, HPG, DH)
            vkeep = np.ascontiguousarray(
                np.concatenate(
                    [vk, np.ones((NC - NX, HPG, 1), np.float32)], axis=2
                ).reshape(NC - NX, HPG * (DH + 1))
            ).astype(ml_dtypes.bfloat16)
            wprojT = np.asarray(_STATE["wprojT"][g], ml_dtypes.bfloat16)
            in_maps.append(
                dict(
                    xT=xT, wqkT=wqkT, wvT=wvT, bqk=bqk, bv=bv,
                    kkeepT=kkeepT, vkeep=vkeep, wprojT=wprojT,
                    ones8=np.ones((128, 8), ml_dtypes.bfloat16),
                    id128=np.eye(128, dtype=ml_dtypes.bfloat16),
                )
            )
    return in_maps


def kernel(x, update_idx, cache_k, cache_v, w_qkv, b_qkv, w_proj, b_proj):
    nc = _get_nc()
    w_proj = np.asarray(w_proj, np.float32)
    b_proj = np.asarray(b_proj, np.float32)
    _STATE["wprojT"] = [
        np.ascontiguousarray(w_proj[:, g * CG:(g + 1) * CG].T) for g in range(G)
    ]
    in_maps = _prep_in_maps(x, update_idx, cache_k, cache_v, w_qkv, b_qkv)
    res = bass_utils.run_bass_kernel_spmd(nc, in_maps, core_ids=list(range(N_CORES)))
    _STATE["last_results"] = res
    out = np.empty((B, NX, C), np.float32)
    for b in range(B):
        acc = (res.results[2 * b]["outT"].astype(np.float32)
               + res.results[2 * b + 1]["outT"].astype(np.float32))
        out[b] = acc.T + b_proj
    return out


# revision 12
# speedup vs baseline: 1.2414x; 1.0925x over previous
"""Decoder attention (QKV proj + KV-cache scatter + full softmax attention + out proj)
on 8 Trainium2 cores.

Sharding: core = (batch b, head-group g).  b = core//2, g = core%2; each core
handles 8 of the 16 heads for one batch element.

Key algorithmic point: softmax + attn@V are invariant to a permutation of the
key axis, so the reference's masked_scatter of new K/V into the cache is
equivalent to attending over concat([k_new, cache_keep]) where cache_keep are
the cache rows NOT in update_idx (complement set, gathered host-side during
sharding).  No on-device scatter is needed.

v2 structure (HW-microbenchmarked):
  - scores run with the contraction PADDED to K=128 (zeros in the k tiles).
    Measured: K=64 matmuls stream at 2 cycles/row on TRN2; K>=96 at 1.
    Each per-head k tile [128, NC] holds its 64 dh rows at the head's own
    partition offset (po) and zeros elsewhere, so the full 128-partition q
    tile (2 heads stacked) can be streamed unmodified as the moving operand.
  - attn@V is TRANSPOSED: out[n,d] chunks [128, 65] with et as stationary and
    the per-head v slice [128 kv, 64+ones] as the 65-row moving operand.
    Measured 279ns vs 435ns per (head, j-tile).  The ones column makes the
    softmax denominator accumulate in column 64 of each chunk.
  - normalization is then per-partition scalars (DVE reciprocal +
    tensor_scalar mult), no partition_broadcast needed.
  - the [n, cg] attention output is PE-transposed (identity moving operand)
    back to [cg, n] for the output projection, 128x128 blocks, interleaved
    into the attention drain stream.
  - fp8/DoubleRow: measured NO gain over bf16 (sc_fp8dr == sc_bf16), not used.

All matmuls bf16 (fp32 PSUM accumulation).
"""

import sys

import os

for _p in ("/opt/trn_rl_repo", "/root/.axon_site/_ro/trn_rl_repo"):
    if os.path.isdir(_p) and _p not in sys.path:
        sys.path.insert(0, _p)
        break

import numpy as np

import concourse.bacc as bacc
import concourse.mybir as mybir
import concourse.tile as tile
from concourse import bass_utils

B, NX, NC, C, H = 4, 1024, 2048, 1024, 16
DH = C // H                      # 64
G = 2                            # head groups (tensor-parallel factor)
HPG = H // G                     # 8 heads per group
CG = HPG * DH                    # 512 channels per group
SCALE = DH ** -0.5
N_CORES = 8
F32 = mybir.dt.float32
BF16 = mybir.dt.bfloat16
EXP = mybir.ActivationFunctionType.Exp
IDENT = mybir.ActivationFunctionType.Identity

_STATE = {}


def _build(reps: int = 1, exp_mode: str = "act"):
    """Build + compile the per-core Bass program.

    reps > 1 wraps the whole computation in an on-device hardware loop --
    used only for timing (amortizes host->device dispatch latency).
    """
    nc = bacc.Bacc("TRN2", target_bir_lowering=False, debug=False)

    xT_d = nc.dram_tensor("xT", [C, NX], BF16, kind="ExternalInput")
    wqkT_d = nc.dram_tensor("wqkT", [C, 2 * CG], BF16, kind="ExternalInput")
    wvT_d = nc.dram_tensor("wvT", [C, CG], BF16, kind="ExternalInput")
    bqk_d = nc.dram_tensor("bqk", [128, 8], F32, kind="ExternalInput")
    bv_d = nc.dram_tensor("bv", [128, CG], F32, kind="ExternalInput")
    kkeepT_d = nc.dram_tensor("kkeepT", [CG, NC - NX], BF16, kind="ExternalInput")
    vkeep_d = nc.dram_tensor("vkeep", [NC - NX, HPG * (DH + 1)], BF16, kind="ExternalInput")
    wprojT_d = nc.dram_tensor("wprojT", [CG, C], BF16, kind="ExternalInput")
    ones_d = nc.dram_tensor("ones8", [128, 8], BF16, kind="ExternalInput")
    id_d = nc.dram_tensor("id128", [128, 128], BF16, kind="ExternalInput")
    outT_d = nc.dram_tensor("outT", [C, NX], BF16, kind="ExternalOutput")

    NJ = NC // 128               # 16 j-tiles over the effective kv length
    VW = DH + 1                  # 65: v columns + ones column per head
    NT = NX // 128               # 8 query n-tiles

    with tile.TileContext(nc) as tc:
        with (
            tc.tile_pool(name="persist", bufs=1) as pp,
            tc.tile_pool(name="work", bufs=1) as wp,
            tc.tile_pool(name="wqkc", bufs=4) as wqkp,
            tc.tile_pool(name="attn", bufs=4) as ep,
            tc.tile_pool(name="nrm", bufs=2) as np_pool,
            tc.tile_pool(name="out_sb", bufs=1) as op,
            tc.tile_pool(name="ps", bufs=1, space="PSUM") as psp,
        ):
            # ---- persistent tiles ----
            q_t = [pp.tile([128, NX], BF16, tag=f"q{i}", name=f"q{i}") for i in range(4)]
            # per-head k tiles, dh rows at partition offset po(h), zeros
            # elsewhere (K=128 padding for full-rate scores matmuls)
            k_t = [pp.tile([128, NC], BF16, tag=f"k{h}", name=f"k{h}") for h in range(HPG)]
            v_t = [pp.tile([128, HPG * VW], BF16, tag=f"v{i}", name=f"v{i}") for i in range(NJ)]
            # attention out, transposed layout: aT_t[nt] = [128 n, CG] bf16
            aT_t = [pp.tile([128, CG], BF16, tag=f"aT{i}", name=f"aT{i}") for i in range(NT)]
            # attention out, proj layout: a_t[c] = [128 cg-chunk, NX]
            a_t = [pp.tile([128, NX], BF16, tag=f"a{i}", name=f"a{i}") for i in range(4)]
            id_t = pp.tile([128, 128], BF16, tag="id128")
            bqk_t = pp.tile([128, 8], F32, tag="bqk")
            bv_t = pp.tile([128, CG], F32, tag="bv")
            xT_t = [wp.tile([128, NX], BF16, tag=f"x{i}", name=f"x{i}") for i in range(8)]
            wv_t = [wp.tile([128, CG], BF16, tag=f"wv{i}", name=f"wv{i}") for i in range(8)]

            # zero the k pad rows once (outside the timing loop; the loop only
            # ever rewrites the dh rows, so the pad stays zero across reps).
            # On GPSIMD: the Pool engine is idle in the prologue, while DVE
            # memsets would delay the first qk bias evacuations behind them.
            for h in range(HPG):
                po = DH * (h % 2)
                pad0 = 0 if h % 2 == 1 else DH
                nc.gpsimd.memset(k_t[h][pad0:pad0 + DH, :], 0.0)

            nc.sync.dma_start(bqk_t[:], bqk_d.ap())
            nc.sync.dma_start(bv_t[:], bv_d.ap())
            nc.sync.dma_start(id_t[:], id_d.ap())

            def body():
                # ACT exp-table preload: a tiny dummy exp up front makes the
                # ~2.7us ACT_TABLE_LOAD happen during the DMA-bound prologue
                # instead of delaying the first real softmax exp.
                scr0 = wp.tile([128, 8], F32, tag="scr0")
                scr1 = wp.tile([128, 8], F32, tag="scr1")
                nc.vector.memset(scr0[:], 0.0)
                nc.scalar.activation(scr1[:], scr0[:], EXP)

                # ---- priority DMAs, ordered by first use.  The m=0/m=4
                # weight batches go out FIRST (right after xT chunk 0) so the
                # prologue qk matmuls are paced by the xT stream, not stuck
                # behind ~5MB of v/k-cache input traffic.
                nc.sync.dma_start(xT_t[0][:], xT_d[0:128, :])
                wqk_pre = {}
                for m in (0, 4):
                    wqk_pre[m] = wqkp.tile([128, 1024], BF16, tag="wqkc",
                                           bufs=3, name=f"wqkm{m}")
                    nc.sync.dma_start(
                        wqk_pre[m][:].rearrange("p (kk c) -> p kk c", kk=8),
                        wqkT_d[0:C, m * 128:(m + 1) * 128]
                        .rearrange("(kk p) c -> p kk c", p=128),
                    )
                # kkeep for heads 0,1 (head h dh rows at partition offset po)
                for h in (0, 1):
                    po = DH * (h % 2)
                    nc.sync.dma_start(
                        k_t[h][po:po + DH, NX:NC],
                        kkeepT_d[h * DH:(h + 1) * DH, :],
                    )
                for i in range(1, 8):
                    nc.sync.dma_start(xT_t[i][:], xT_d[i * 128:(i + 1) * 128, :])
                # ones columns of the v tiles (bias adds never touch them;
                # vkeep rows arrive with ones baked in)
                for m in range(NJ // 2):
                    nc.sync.dma_start(
                        v_t[m][:].rearrange("p (h w) -> p h w", w=VW)[:, :, DH],
                        ones_d.ap(),
                    )
                for i in range(8):
                    nc.sync.dma_start(wv_t[i][:], wvT_d[i * 128:(i + 1) * 128, :])
                for h in range(2, HPG):
                    po = DH * (h % 2)
                    nc.sync.dma_start(
                        k_t[h][po:po + DH, NX:NC],
                        kkeepT_d[h * DH:(h + 1) * DH, :],
                    )
                for j in range(NJ // 2, NJ):
                    r0 = (j - NJ // 2) * 128
                    nc.sync.dma_start(v_t[j][:], vkeep_d[r0:r0 + 128, :])
                # prefetch the output-projection weights now: the DMA engines
                # go idle once the priority inputs land, and fetching wproj
                # inside the proj loop exposes ~3-4us of Ldweights stalls.
                wp_t = []
                for m in range(8):
                    wp_m = op.tile([128, 512], BF16, tag=f"wpc{m}", bufs=1,
                                   name=f"wpm{m}")
                    wp_t.append(wp_m)
                    nc.sync.dma_start(
                        wp_m[:].rearrange("p (kk c) -> p kk c", kk=4),
                        wprojT_d[0:CG, m * 128:(m + 1) * 128]
                        .rearrange("(kk p) c -> p kk c", p=128),
                    )

                def qk_thunks(i):
                    """Matmul/bias thunks for q m-tile i and k m-tile 4+i,
                    drained one per attention j-step.  qps is a single PSUM
                    bank [128, 512]; each 512-wide cch chunk is its own
                    matmul chain + DVE evacuation."""
                    for m in (i, 4 + i):
                        if m in wqk_pre:
                            wqk_m = wqk_pre[m]
                        else:
                            wqk_m = wqkp.tile([128, 1024], BF16, tag="wqkc",
                                              bufs=3, name=f"wqkm{m}")

                            def wdma(m=m, wqk_m=wqk_m):
                                nc.sync.dma_start(
                                    wqk_m[:].rearrange("p (kk c) -> p kk c",
                                                       kk=8),
                                    wqkT_d[0:C, m * 128:(m + 1) * 128]
                                    .rearrange("(kk p) c -> p kk c", p=128),
                                )
                            yield wdma
                        for cch in range(2):
                            qps = psp.tile([128, 512], F32, tag="qps", bufs=1,
                                           name=f"qps{m}_{cch}")
                            for kk in range(8):
                                def mm(m=m, kk=kk, cch=cch, qps=qps, wqk_m=wqk_m):
                                    nc.tensor.matmul(
                                        qps[:],
                                        wqk_m[:, kk * 128:(kk + 1) * 128],
                                        xT_t[kk][:, cch * 512:(cch + 1) * 512],
                                        start=(kk == 0),
                                        stop=(kk == 7),
                                    )
                                yield mm
                            def bias(m=m, cch=cch, qps=qps):
                                cs = slice(cch * 512, (cch + 1) * 512)
                                if m < 4:
                                    nc.vector.tensor_scalar_add(
                                        q_t[m][:, cs], qps[:], bqk_t[:, m:m + 1])
                                else:
                                    i2 = m - 4
                                    # two heads: dh rows land at each head's po
                                    nc.vector.tensor_scalar_add(
                                        k_t[2 * i2][0:DH, cs], qps[0:DH, :],
                                        bqk_t[0:DH, m:m + 1])
                                    nc.vector.tensor_scalar_add(
                                        k_t[2 * i2 + 1][DH:128, cs], qps[DH:128, :],
                                        bqk_t[DH:128, m:m + 1])
                            yield bias

                pending = []

                def drain(n):
                    for _ in range(n):
                        if not pending:
                            return
                        pending.pop(0)()

                # qk pair 0 runs up front (attention depends on it)
                for th in qk_thunks(0):
                    th()

                def v_thunks():
                    """v projection m-tiles as drain thunks (9 per m: 8 matmuls
                    + one batched strided bias/scatter finisher)."""
                    for m in range(8):
                        vps = psp.tile([128, 512], F32, tag="qps", bufs=1,
                                       name=f"vps{m}")
                        for kk in range(8):
                            def mm(m=m, kk=kk, vps=vps):
                                nc.tensor.matmul(
                                    vps[:],
                                    xT_t[kk][:, m * 128:(m + 1) * 128],
                                    wv_t[kk][:],
                                    start=(kk == 0),
                                    stop=(kk == 7),
                                )
                            yield mm
                        def fin(m=m, vps=vps):
                            nc.vector.tensor_add(
                                v_t[m][:].rearrange("p (h w) -> p h w", w=VW)
                                [:, :, 0:DH],
                                vps[:].rearrange("p (h w) -> p h w", w=DH),
                                bv_t[:].rearrange("p (h w) -> p h w", w=DH),
                            )
                        yield fin

                def transpose_thunks(c):
                    """PE-transpose aT_t[:, c*128:(c+1)*128] -> a_t[c], in two
                    4-block batches through one PSUM bank."""
                    for half in range(2):
                        trp = psp.tile([128, 512], BF16, tag="tr", bufs=1,
                                       name=f"tr{c}_{half}")
                        for q in range(4):
                            nt = half * 4 + q
                            def tmm(c=c, q=q, nt=nt, trp=trp):
                                nc.tensor.matmul(
                                    trp[:, q * 128:(q + 1) * 128],
                                    aT_t[nt][:, c * 128:(c + 1) * 128],
                                    id_t[:],
                                    start=(q == 0), stop=(q == 3),
                                    is_transpose=True,
                                )
                            yield tmm
                        def tev(c=c, half=half, trp=trp):
                            nc.vector.tensor_copy(
                                a_t[c][:, half * 512:(half + 1) * 512], trp[:])
                        yield tev

                # ---- phase 2: attention; per j-step: scores (K=128 padded),
                # exp on ACT, transposed av for the previous j (8 chunk
                # matmuls), plus drained QKV/transpose thunks to fill PE ----
                def attn_head(h, av_lag=4, drain_per_j=1, drain_first=4,
                              j_order=None):
                    hp = h // 2
                    jo = list(j_order) if j_order is not None else list(range(NJ))
                    avA = psp.tile([128, 4 * VW], F32, tag="avA", bufs=1,
                                   name=f"avA{h}")
                    avB = psp.tile([128, 4 * VW], F32, tag="avB", bufs=1,
                                   name=f"avB{h}")
                    ets = [None] * NJ

                    def emit_av(j):
                        # one accumulation group per PSUM bank (2KB zero
                        # region): start only on the bank's first write,
                        # stop on its last; per-element first-touch handles
                        # the four disjoint nt slices within the bank.
                        for nt in range(NT):
                            dst = avA if nt < 4 else avB
                            o = (nt % 4) * VW
                            nc.tensor.matmul(
                                dst[:, o:o + VW],
                                ets[j][:, nt * 128:(nt + 1) * 128],
                                v_t[j][:, h * VW:(h + 1) * VW],
                                start=(j == jo[0] and nt % 4 == 0),
                                stop=(j == jo[-1] and nt % 4 == 3),
                            )

                    for step, j in enumerate(jo):
                        sps = psp.tile([128, NX], F32, tag="sps", bufs=2, name=f"sps{h}_{j}")
                        for cch in range(2):
                            nc.tensor.matmul(
                                sps[:, cch * 512:(cch + 1) * 512],
                                k_t[h][:, j * 128:(j + 1) * 128],
                                q_t[hp][:, cch * 512:(cch + 1) * 512],
                                start=True,
                                stop=True,
                            )
                        et = ep.tile([128, NX], BF16, tag="et", bufs=6,
                                     name=f"et{h}_{j}")
                        ets[j] = et
                        if exp_mode == "act":
                            nc.scalar.activation(et[:], sps[:], EXP, scale=SCALE)
                        else:
                            # timing probe only: wrong math, same data movement
                            nc.vector.tensor_copy(et[:], sps[:])
                        drain(drain_first if step < 4 else drain_per_j)
                        if step >= av_lag:
                            emit_av(jo[step - av_lag])
                    for step in range(NJ - av_lag, NJ):
                        emit_av(jo[step])
                    # normalization: per n-tile, denominator is column 64 of
                    # the av chunk; divide via per-partition scalars on DVE.
                    # Emitted as drain thunks: issuing these eagerly would
                    # head-of-line-block the DVE queue on the av-stop matmuls
                    # at every head boundary.  av_lag=4 + drain_first=4 makes
                    # the next head's first av matmul (WAR on avA/avB) wait
                    # only on already-drained work.
                    def norm_thunks(h=h, avA=avA, avB=avB):
                        for nt in range(NT):
                            def nrm(h=h, nt=nt, avA=avA, avB=avB):
                                src = avA if nt < 4 else avB
                                o = (nt % 4) * VW
                                rc = np_pool.tile([128, 1], F32, tag="rc",
                                                  bufs=2, name=f"rc{h}_{nt}")
                                nc.vector.reciprocal(
                                    rc[:], src[:, o + DH:o + DH + 1])
                                nc.vector.tensor_scalar_mul(
                                    aT_t[nt][:, h * DH:(h + 1) * DH],
                                    src[:, o:o + DH], rc[:])
                            yield nrm
                    pending.extend(norm_thunks())

                pending.extend(v_thunks())
                # head 0 visits the cache-half key tiles first: their V rows
                # come from DMA, so attention starts before the V projection
                # (draining concurrently) has produced anything
                attn_head(0, av_lag=4, drain_per_j=9, drain_first=9,
                          j_order=list(range(NJ // 2, NJ)) + list(range(NJ // 2)))
                for h in range(1, HPG):
                    if h in (2, 4, 6):
                        drain(90)            # pair (h//2) must be complete
                    if h in (1, 3, 5):
                        pending.extend(qk_thunks((h + 1) // 2))
                    if h in (2, 4, 6):
                        pending.extend(transpose_thunks(h // 2 - 1))
                    attn_head(h)
                pending.extend(transpose_thunks(3))
                drain(200)

                # ---- phase 3: output projection (partial: this head group);
                # w_proj column-chunks streamed per m like w_qk ----
                for m in range(8):
                    pps = psp.tile([128, NX], F32, tag="sps", bufs=2, name=f"pps{m}")
                    wp_m = wp_t[m]
                    for cch in range(2):
                        for kk in range(4):
                            nc.tensor.matmul(
                                pps[:, cch * 512:(cch + 1) * 512],
                                wp_m[:, kk * 128:(kk + 1) * 128],
                                a_t[kk][:, cch * 512:(cch + 1) * 512],
                                start=(kk == 0),
                                stop=(kk == 3),
                            )
                    ot = op.tile([128, NX], BF16, tag="ot", bufs=2)
                    if m % 2 == 0:
                        nc.vector.tensor_copy(ot[:], pps[:])
                    else:
                        # ACT is idle during proj; split PSUM evacuation
                        nc.scalar.activation(ot[:], pps[:], IDENT)
                    nc.sync.dma_start(outT_d[m * 128:(m + 1) * 128, :], ot[:])

            if reps == 1:
                body()
            else:
                hints = (
                    mybir.EngineType.PE,
                    mybir.EngineType.Activation,
                    mybir.EngineType.DVE,
                    mybir.EngineType.SP,
                )
                with tc.For_i(0, reps, 1, hint_engines=hints):
                    body()

    nc.compile()
    return nc


def _get_nc():
    if "nc" not in _STATE:
        _STATE["nc"] = _build()
    return _STATE["nc"]


def _prep_in_maps(x, update_idx, cache_k, cache_v, w_qkv, b_qkv):
    """Host-side sharding: build the 8 per-core input dicts."""
    import ml_dtypes

    x = np.asarray(x, np.float32)
    update_idx = np.asarray(update_idx)
    cache_k = np.asarray(cache_k, np.float32)
    cache_v = np.asarray(cache_v, np.float32)
    w_qkv = np.asarray(w_qkv, np.float32)
    b_qkv = np.asarray(b_qkv, np.float32)

    per_g = []
    for g in range(G):
        qs = slice(g * CG, (g + 1) * CG)
        ks = slice(C + g * CG, C + (g + 1) * CG)
        vs = slice(2 * C + g * CG, 2 * C + (g + 1) * CG)
        wqkT = np.ascontiguousarray(
            np.concatenate([w_qkv[qs], w_qkv[ks]], 0).T
        ).astype(ml_dtypes.bfloat16)                                                    # (C, 2CG)
        wvT = np.ascontiguousarray(w_qkv[vs].T).astype(ml_dtypes.bfloat16)              # (C, CG)
        bqk = np.ascontiguousarray(
            np.concatenate([b_qkv[qs], b_qkv[ks]]).reshape(8, 128).T
        )                                                    # (128, 8)
        bv = np.broadcast_to(b_qkv[vs][None, :], (128, CG)).copy()
        per_g.append((wqkT, wvT, bqk, bv))

    in_maps = []
    for b in range(B):
        idx = update_idx[b]
        mask = np.ones(NC, bool)
        mask[idx] = False
        keep = np.nonzero(mask)[0]                           # (NC-NX,) sorted
        xT = np.ascontiguousarray(x[b].T).astype(ml_dtypes.bfloat16)                    # (C, NX)
        for g in range(G):
            wqkT, wvT, bqk, bv = per_g[g]
            hsel = slice(g * HPG, (g + 1) * HPG)
            kk = cache_k[b, hsel][:, keep, :]                # (HPG, NC-NX, DH)
            kkeepT = np.ascontiguousarray(
                kk.transpose(0, 2, 1).reshape(HPG * DH, NC - NX)
            ).astype(ml_dtypes.bfloat16)
            vk = cache_v[b, hsel][:, keep, :].transpose(1, 0, 2)  # (NC-NX, HPG, DH)
            vkeep = np.ascontiguousarray(
                np.concatenate(
                    [vk, np.ones((NC - NX, HPG, 1), np.float32)], axis=2
                ).reshape(NC - NX, HPG * (DH + 1))
            ).astype(ml_dtypes.bfloat16)
            wprojT = np.asarray(_STATE["wprojT"][g], ml_dtypes.bfloat16)
            in_maps.append(
                dict(
                    xT=xT, wqkT=wqkT, wvT=wvT, bqk=bqk, bv=bv,
                    kkeepT=kkeepT, vkeep=vkeep, wprojT=wprojT,
                    ones8=np.ones((128, 8), ml_dtypes.bfloat16),
                    id128=np.eye(128, dtype=ml_dtypes.bfloat16),
                )
            )
    return in_maps


def kernel(x, update_idx, cache_k, cache_v, w_qkv, b_qkv, w_proj, b_proj):
    nc = _get_nc()
    w_proj = np.asarray(w_proj, np.float32)
    b_proj = np.asarray(b_proj, np.float32)
    _STATE["wprojT"] = [
        np.ascontiguousarray(w_proj[:, g * CG:(g + 1) * CG].T) for g in range(G)
    ]
    in_maps = _prep_in_maps(x, update_idx, cache_k, cache_v, w_qkv, b_qkv)
    res = bass_utils.run_bass_kernel_spmd(nc, in_maps, core_ids=list(range(N_CORES)))
    _STATE["last_results"] = res
    out = np.empty((B, NX, C), np.float32)
    for b in range(B):
        acc = (res.results[2 * b]["outT"].astype(np.float32)
               + res.results[2 * b + 1]["outT"].astype(np.float32))
        out[b] = acc.T + b_proj
    return out
